# revision 13
# baseline (speedup 1.0000x reference)
"""Trainium2 Bass kernel for nn_EquiCtsConvBase (equivariant continuous conv).

Math (per batch b, center m, field point n):
  rel = (field[n] - center[m]) / RADIUS;  r, theta = polar(rel)
  Bilinear grid-sample of kernel[(co,ci,y,x), theta_pad, r] decomposes into
  separable hats: Wx[j] = relu(1-|4r-0.5-j|), Wy[l] = relu(1-|iy-l|) with
  iy = 4*theta/pi + 4.5 and circular fold of 10 rows -> 8 theta bins.
  att = relu(1-|rel|^2)^3 * mask
  A[cell=(j,b2), n, m] = Wx[j]*att * Wy[b2]
  G[f, cell, m] = sum_n feat[n, f] * A[cell, n, m]          (PE matmul 1)
  out[m, coy]   = sum_{cell,f} G * K2[cell, f, coy]         (PE matmul 2)
  out /= max(psi, tiny), psi[m] = sum_n att (mask col of feat)

v3 schedule notes (vs v0 baseline at ~40us):
  - NO GpSimd compute: DVE and GpSimd arbitrate one shared SBUF port
    pair exclusively per instruction, so concurrent Pool ops serialize
    DVE into 1-port mode (observed lockstep slowdowns).  Pool only
    issues the fkin DMA.
  - r = sqrt(rho + eps) on ACT straight from the PE's rho.  Activation
    func sets thrash-free order: one resident table at a time, so the
    Scalar stream is [set3: sqrt,relu,square] -> load set2 -> [arctan,
    abs, copy ...].
  - theta: iy = phi0*P + Q with P,Q prep interleaved; Wy slot distances
    for all 8 bins in ONE batched TT against fp16 constants in fkin,
    one batched abs, two fold mins.  Hats relu'd on ACT (dedicated
    port): w = relu(1 - e) via scale=-1, bias=1.
  - single aliased PSUM tile [128,8,512]: 8 banks so all 8 matmul1
    groups accumulate independently; copies never stall the PE.
  - PE kept continuously busy (rho -> warm f32r matmuls on pe_s ->
    gpsi -> matmul1) to hold the 2.4GHz p-state (3us ramp); pe_s
    free-dim 480 >= 256 so f32r streams at 1 cycle/row.
  - a_t cell-major [128, cell, u, m]: contiguous DVE writes (the fast
    2x fp16 path needs large contiguous runs).
  - input DMAs issued bias-then-pein on the Scalar queue (observed to
    stream with ~0 start latency vs ~2us on sync).

Sharding: 8 cores; core c handles batch b = c//4, centers m0 = (c%4)*96.
"""

import math
import numpy as np

RADIUS = 1.5
B, M, N = 2, 384, 384
CI = CO = 8
M_LOC = 96          # centers per core
NCH = 3             # n-chunks of 128 (N = 384)
NCELL = 32          # cell = j*8 + b2  (j = radius 0..3, b2 = theta bin 0..7)
N_CORES = 8
PI = math.pi
SQB = 1e-5          # sqrt bias: guards f32r cancellation negatives in rho

CFG = dict(warm=24)

_module_cache = {}

# fkin fp16 column layout
FEAT0 = 0                 # 3 chunks * 17 (16 feat*mask, mask)
K20 = FEAT0 + 3 * 17      # 51: k2c, 4 j * 16 coy
LC8 = K20 + 64            # 115: Wy slot centers 1..8 as fp16
FKW = LC8 + 8             # 123

# bias f32 column layout: activation-bias constants, then coordinates
BIAS_VALS = [1.0, SQB, -0.5, -1.5, -2.5, -3.5]
BIAS_COL = {v: i for i, v in enumerate(BIAS_VALS)}
FX0 = len(BIAS_VALS)      # fx per chunk: 3 cols (then fy: 3 cols)
CX0 = FX0 + 6             # cx per m: 96 cols (then cy: 96)
BIASW = CX0 + 192


def _build_module(cfg):
    import concourse.bass as bass
    import concourse.bacc as bacc
    import concourse.mybir as mybir
    from concourse import tile

    dt = mybir.dt
    Alu = mybir.AluOpType
    Act = mybir.ActivationFunctionType

    nc = bacc.Bacc("TRN2", target_bir_lowering=False, debug=False,
                   num_devices=N_CORES)

    f32 = dt.float32
    f16 = dt.float16
    i16 = dt.int16
    W = cfg["warm"]

    # ------------- DRAM I/O -------------
    ped = nc.dram_tensor("pein", [4, 480], f32, kind="ExternalInput").ap()
    biasd = nc.dram_tensor("bias", [128, BIASW], f32,
                           kind="ExternalInput").ap()
    fkd = nc.dram_tensor("fkin", [128, FKW], f16, kind="ExternalInput").ap()
    outd = nc.dram_tensor("out", [M_LOC, 16], f32, kind="ExternalOutput").ap()

    with tile.TileContext(nc) as tc:
        with tc.tile_pool(name="p", bufs=1) as pool, \
             tc.tile_pool(name="ps", bufs=1, space="PSUM") as psum:

            V, S, G = nc.vector, nc.scalar, nc.gpsimd

            # ---------- SBUF tiles ----------
            fk_s = pool.tile([128, FKW], f16, tag="fk", name="fk_s")
            bias_s = pool.tile([128, BIASW], f32, tag="bias", name="bias_s")
            pe_s = pool.tile([4, 480], f32, tag="pein", name="pe_s")

            def bAP(v, parts=128):
                return bias_s[0:parts, BIAS_COL[v]:BIAS_COL[v] + 1]

            def t16(tag, shape=None):
                return pool.tile(shape or [128, NCH, M_LOC], f16, tag=tag,
                                 name=tag)

            relxy = t16("relxy", [128, 2, NCH, M_LOC])
            axy = t16("axy", [128, 2, NCH, M_LOC])
            sgxy = t16("sgxy", [128, 2, NCH, M_LOC])
            mn = t16("mn")
            mx = pool.tile([128, NCH, M_LOC], f32, tag="mx", name="mx")
            rmx = pool.tile([128, NCH, M_LOC], f32, tag="rmx", name="rmx")
            ratio = t16("ratio")
            phi0 = t16("phi0")
            swp = t16("swp"); qa = t16("qa"); w2 = t16("w2"); Qt = t16("Qt")
            sxy = t16("sxy"); s1p = t16("s1p"); Pt = t16("Pt"); m1 = t16("m1")
            tphi = t16("tphi"); iy = t16("iy"); e9m = t16("e9m")
            r16 = t16("r16")
            ex = t16("ex", [128, 4, NCH, M_LOC])
            ey = t16("ey", [128, 8, NCH, M_LOC])
            wyt = t16("wyt", [128, 8, NCH, M_LOC])
            wxt = t16("wxt", [128, 4, NCH, M_LOC])
            wxa = t16("wxa", [128, 4, NCH, M_LOC])
            au = t16("au"); au2 = t16("au2")
            a_t = pool.tile([128, NCELL + 1, NCH, M_LOC], f16, tag="a_t",
                            name="a_t")
            gs_s = pool.tile([16, NCELL, M_LOC], f16, tag="gs", name="gs_s")
            gt = pool.tile([128, 4, M_LOC], f16, tag="gt", name="gt")
            psir = pool.tile([1, M_LOC], f32, tag="psir", name="psir")
            psit = pool.tile([M_LOC, 1], f32, tag="psit", name="psit")
            out_s = pool.tile([M_LOC, 16], f32, tag="outs", name="out_s")
            warm = pool.tile([1, 1], f32, tag="warm", name="warm")

            # ---------- PSUM: one aliased tile, manual bank layout ----------
            # bank0: rho | gh(0,0)        bank4: gh(2,0)
            # bank1: gh(1,0), o2t@400     bank5: gh(3,0)
            # bank2: warm | gh(2,1)       bank6: gh(0,1)
            # bank3: gpsi | gh(3,1)       bank7: gh(1,1)
            P = psum.tile([128, 8, 512], f32, tag="P", name="P")
            rho_p = P[:, 0, 0:NCH * M_LOC].rearrange(
                "p (u m) -> p u m", u=NCH)
            warm_ps = P[:, 2, 0:480]
            gpsi = P[0:1, 3, 0:M_LOC]
            o2t = P[0:M_LOC, 1, 400:416]
            GH_BANK = {(0, 0): 0, (1, 0): 1, (2, 0): 4, (3, 0): 5,
                       (0, 1): 6, (1, 1): 7, (2, 1): 2, (3, 1): 3}

            def ghv(j, h):
                return P[0:17, GH_BANK[(j, h)], 0:4 * M_LOC].rearrange(
                    "p (c m) -> p c m", c=4)

            # ---------- input DMAs (bias first: streams immediately) ------
            nc.scalar.dma_start(bias_s[:], biasd[:])
            nc.scalar.dma_start(pe_s[:], ped[:])
            nc.gpsimd.dma_start(fk_s[:], fkd[:])

            # act-set seed: Sqrt first-fits set 3 ({sqrt,relu,square,abs});
            # single switch to set 2 (arctan/abs/copy) before the arctan.
            V.memset(warm[:], 0.25)
            S.activation(warm[:], warm[:], Act.Sqrt)

            # ---------- PE front: rho -> PSUM, then p-state warm ----------
            for u in range(NCH):
                nc.tensor.matmul(
                    rho_p[:, u, :],
                    pe_s[0:4, u * 128:(u + 1) * 128],
                    pe_s[0:4, 384:480])
            for _ in range(3):
                nc.tensor.matmul(warm_ps[:], pe_s[0:4, 0:128],
                                 pe_s[0:4, 0:480])

            # ---------- rel (f32 coords -> fp16) ----------
            f_b = bias_s[:, FX0:FX0 + 6, None].rearrange(
                "p (a u) o -> p a u o", a=2).to_broadcast(
                (128, 2, NCH, M_LOC))
            c_b = bias_s[:, None, CX0:CX0 + 192].rearrange(
                "p o (a m) -> p a o m", a=2).to_broadcast(
                (128, 2, NCH, M_LOC))
            V.tensor_tensor(relxy[:], f_b, c_b, Alu.subtract)

            # ---------- octant pieces (all DVE) ----------
            V.tensor_scalar(axy[:].bitcast(i16), relxy[:].bitcast(i16),
                            0x7FFF, None, Alu.bitwise_and)
            V.tensor_scalar(sgxy[:].bitcast(i16), relxy[:].bitcast(i16),
                            -32768, 0x3C00, Alu.bitwise_and, Alu.bitwise_or)
            ax, ay = axy[:, 0], axy[:, 1]
            sgx, sgy = sgxy[:, 0], sgxy[:, 1]
            V.tensor_tensor(mn[:], ax, ay, Alu.min)
            V.tensor_tensor(mx[:], ax, ay, Alu.max)
            V.tensor_tensor(swp[:], ay, ax, Alu.is_gt)
            V.reciprocal_approx_fast(rmx[:], mx[:])
            V.tensor_tensor(ratio[:], mn[:], rmx[:], Alu.mult)

            # theta prep (parallel with arctan):
            #   iy = phi0 * P + Q
            #   P = sxy * (4/pi) * (1 - 2*swp)
            #   Q = sxy * (2*swp - 2) + (2*sgy + 4.5)
            V.tensor_tensor(sxy[:], sgx, sgy, Alu.mult)
            V.tensor_scalar(s1p[:], swp[:], -8.0 / PI, 4.0 / PI,
                            Alu.mult, Alu.add)
            V.tensor_tensor(Pt[:], s1p[:], sxy[:], Alu.mult)
            V.tensor_scalar(qa[:], swp[:], 2.0, -2.0, Alu.mult, Alu.add)
            V.tensor_tensor(m1[:], qa[:], sxy[:], Alu.mult)
            V.tensor_scalar(w2[:], sgy, 2.0, 4.5, Alu.mult, Alu.add)
            V.tensor_tensor(Qt[:], m1[:], w2[:], Alu.add)

            # ---------- Scalar chain ----------
            S.activation(r16[:], rho_p[:], Act.Sqrt, bias=bAP(SQB))
            S.activation(au[:], rho_p[:], Act.Relu, bias=bAP(1.0), scale=-1.0)
            S.activation(au2[:], au[:], Act.Square)
            S.activation(phi0[:], ratio[:], Act.Arctan)
            # Wx slots: |4r - (0.5+j)|
            for j in range(4):
                S.activation(ex[:, j, :, :], r16[:], Act.Abs,
                             bias=bAP(-0.5 - j), scale=4.0)
            # wxt = relu(1 - ex) on ACT (dedicated port)
            S.activation(wxt[:], ex[:], Act.Relu, bias=bAP(1.0), scale=-1.0)

            # att -> a_t att cell
            V.tensor_tensor(a_t[:, NCELL, :, :], au2[:], au[:], Alu.mult)

            # ---------- theta tail + batched Wy slots ----------
            V.tensor_tensor(tphi[:], phi0[:], Pt[:], Alu.mult)
            V.tensor_tensor(iy[:], tphi[:], Qt[:], Alu.add)
            V.tensor_scalar(e9m[:], iy[:], -1.0, 9.0, Alu.mult, Alu.add)
            # all 8 slot distances in one op: ey[b] = iy - (b+1)
            iy_b = iy[:, None, :, :].to_broadcast((128, 8, NCH, M_LOC))
            l8_b = fk_s[:, LC8:LC8 + 8, None, None].to_broadcast(
                (128, 8, NCH, M_LOC))
            V.tensor_tensor(ey[:], iy_b, l8_b, Alu.subtract)
            V.tensor_scalar(ey[:].bitcast(i16), ey[:].bitcast(i16),
                            0x7FFF, None, Alu.bitwise_and)
            # circular folds: |iy-9| = 9-iy, |iy-0| = iy  (iy in [0.5, 8.5])
            V.tensor_tensor(ey[:, 0, :, :], ey[:, 0, :, :], e9m[:], Alu.min)
            V.tensor_tensor(ey[:, 7, :, :], ey[:, 7, :, :], iy[:], Alu.min)
            # wyt = relu(1 - ey) on ACT, in halves to unblock A-products
            S.activation(wyt[:, 0:4, :, :], ey[:, 0:4, :, :], Act.Relu,
                         bias=bAP(1.0), scale=-1.0)
            S.activation(wyt[:, 4:8, :, :], ey[:, 4:8, :, :], Act.Relu,
                         bias=bAP(1.0), scale=-1.0)

            # wxa = wxt * att
            att_b = a_t[:, NCELL:NCELL + 1, :, :].to_broadcast(
                (128, 4, NCH, M_LOC))
            V.tensor_tensor(wxa[:], wxt[:], att_b, Alu.mult)

            # ---------- gpsi (psi) + warm chain ----------
            def feat_ap(u):
                return fk_s[:, FEAT0 + 17 * u:FEAT0 + 17 * (u + 1)]

            # psi via the mask column only -> partition 0 of bank 3
            for u in range(NCH):
                nc.tensor.matmul(gpsi[:],
                                 fk_s[:, FEAT0 + 17 * u + 16:
                                      FEAT0 + 17 * u + 17],
                                 a_t[:, NCELL, u, :],
                                 start=(u == 0), stop=(u == NCH - 1))
            for _ in range(W - 3):
                nc.tensor.matmul(warm_ps[:], pe_s[0:4, 0:128],
                                 pe_s[0:4, 0:480])

            # psi -> 1/psi -> [96, 1]
            V.tensor_scalar(psir[:], gpsi[0:1, :], 1e-35, None, Alu.max)
            V.reciprocal_approx_fast(psir[:], psir[:])
            nc.sync.dma_start(psit[:, 0:1], psir[0:1, :])

            # ---------- A cells + matmul1 + copies + transpose DMAs -------
            def wxa_b(j):
                return wxa[:, j:j + 1, :, :].to_broadcast(
                    (128, 8, NCH, M_LOC))

            def mm1(j, h):
                g = ghv(j, h)
                c0 = 8 * j + 4 * h
                for u in range(NCH):
                    nc.tensor.matmul(g[:], feat_ap(u),
                                     a_t[:, c0:c0 + 4, u, :],
                                     start=(u == 0), stop=(u == NCH - 1))

            def gcopy(j, h, eng):
                c0 = 8 * j + 4 * h
                src = ghv(j, h)[0:16]
                if eng is S:
                    S.activation(gs_s[:, c0:c0 + 4, :], src, Act.Copy)
                else:
                    eng.tensor_copy(gs_s[:, c0:c0 + 4, :], src)

            for j in range(4):
                # one 8-cell product per j on DVE (fast contiguous pattern)
                V.tensor_tensor(a_t[:, 8 * j:8 * j + 8, :, :], wxa_b(j),
                                wyt[:], Alu.mult)
                mm1(j, 0)
                mm1(j, 1)
                ceng = S if j < 3 else V
                gcopy(j, 0, ceng)
                gcopy(j, 1, ceng)
                nc.sync.dma_start(gt[:, j], gs_s[:, 8 * j:8 * j + 8, :])

            # ---------- matmul2 ----------
            for q in range(4):
                nc.tensor.matmul(o2t[:], gt[:, q, :],
                                 fk_s[:, K20 + 16 * q:K20 + 16 * (q + 1)],
                                 start=(q == 0), stop=(q == 3))

            # ---------- scale by 1/psi, store ----------
            V.tensor_scalar(out_s[:], o2t[:], psit[:, 0:1], None, Alu.mult)
            nc.scalar.dma_start(outd[:], out_s[:])

    nc.compile()
    return nc


def get_module(cfg=None):
    cfg = dict(CFG, **(cfg or {}))
    key = tuple(sorted((k, str(v)) for k, v in cfg.items()))
    if key not in _module_cache:
        _module_cache[key] = _build_module(cfg)
    return _module_cache[key]


def make_in_maps(field, center, field_feat, field_mask, kernel, cfg=None):
    """Host-side shard + layout prep. Returns list of 8 in_maps."""
    field = np.asarray(field, np.float32)
    center = np.asarray(center, np.float32)
    feat = np.asarray(field_feat, np.float32)
    mask = np.asarray(field_mask, np.float32)
    ker = np.asarray(kernel, np.float32)

    # kk[cell=(th*4+r), f=(ci,x), coy=(co,y)]
    kk = ker.transpose(3, 2, 1, 5, 0, 4).reshape(NCELL, 16, 16)
    # k2c[p=(f*8+b2), j, coy] = kk[b2*4+j, f]
    k2c = np.zeros((128, 4, 16), np.float32)
    for bth in range(8):
        for j in range(4):
            for f in range(16):
                k2c[f * 8 + bth, j] = kk[bth * 4 + j, f]

    in_maps = []
    for c in range(N_CORES):
        b, blk = divmod(c, 4)
        m0 = blk * M_LOC
        cx = center[b, m0:m0 + M_LOC, 0] / RADIUS   # [96]
        cy = center[b, m0:m0 + M_LOC, 1] / RADIUS
        fx = (field[b, :, 0] / RADIUS).reshape(NCH, 128)  # [3, 128]
        fy = (field[b, :, 1] / RADIUS).reshape(NCH, 128)
        ffsq = fx * fx + fy * fy
        ccsq = cx * cx + cy * cy

        pein = np.zeros((4, 480), np.float32)
        pein[0, 0:384] = ffsq.reshape(-1)
        pein[1, 0:384] = fx.reshape(-1)
        pein[2, 0:384] = fy.reshape(-1)
        pein[3, 0:384] = 1.0
        pein[0, 384:480] = 1.0
        pein[1, 384:480] = -2.0 * cx
        pein[2, 384:480] = -2.0 * cy
        pein[3, 384:480] = ccsq

        biasf = np.zeros((128, BIASW), np.float32)
        biasf[:, 0:len(BIAS_VALS)] = np.array(BIAS_VALS, np.float32)
        biasf[:, FX0:FX0 + 3] = fx.T
        biasf[:, FX0 + 3:FX0 + 6] = fy.T
        biasf[:, CX0:CX0 + 96] = cx
        biasf[:, CX0 + 96:CX0 + 192] = cy

        fkin = np.zeros((128, FKW), np.float32)
        fm = feat[b].reshape(N, 16) * mask[b]
        fcols = np.concatenate([fm, mask[b]], axis=1)        # [N, 17]
        fkin[:, FEAT0:FEAT0 + 51] = (
            fcols.reshape(NCH, 128, 17).transpose(1, 0, 2).reshape(128, 51))
        fkin[:, K20:K20 + 64] = k2c.reshape(128, 64)
        fkin[:, LC8:LC8 + 8] = np.arange(1, 9, dtype=np.float32)

        in_maps.append({
            "pein": pein,
            "bias": biasf,
            "fkin": fkin.astype(np.float16),
        })
    return in_maps


def unshard(results):
    out = np.zeros((B, M, CO, 2), np.float32)
    for c in range(N_CORES):
        b, blk = divmod(c, 4)
        m0 = blk * M_LOC
        out[b, m0:m0 + M_LOC] = results[c]["out"].reshape(M_LOC, CO, 2)
    return out


def kernel(field, center, field_feat, field_mask, kernel):
    from concourse.bass_utils import run_bass_kernel_spmd
    nc = get_module()
    in_maps = make_in_maps(field, center, field_feat, field_mask, kernel)
    res = run_bass_kernel_spmd(nc, in_maps, core_ids=list(range(N_CORES)))
    return unshard(res.results)


# revision 16
# speedup vs baseline: 1.7542x; 1.7542x over previous
"""Trainium2 Bass kernel for nn_EquiCtsConvBase (equivariant continuous conv).

Math (per batch b, center m, field point n):
  rel = (field[n] - center[m]) / RADIUS;  r, theta = polar(rel)
  Bilinear grid-sample of kernel[(co,ci,y,x), theta_pad, r] decomposes into
  separable hats: Wx[j] = relu(1-|4r-0.5-j|), Wy[l] = relu(1-|iy-l|) with
  iy = 4*theta/pi + 4.5 and circular fold of 10 rows -> 8 theta bins.
  att = relu(1-|rel|^2)^3 * mask
  A[cell=(j,b2), n, m] = Wx[j]*att * Wy[b2]
  G[f, cell, m] = sum_n feat[n, f] * A[cell, n, m]          (PE matmul 1)
  out[m, coy]   = sum_{cell,f} G * K2[cell, f, coy]         (PE matmul 2)
  out /= max(psi, tiny), psi[m] = sum_n att (mask col of feat)

v3 schedule notes (vs v0 baseline at ~40us):
  - NO GpSimd compute: DVE and GpSimd arbitrate one shared SBUF port
    pair exclusively per instruction, so concurrent Pool ops serialize
    DVE into 1-port mode (observed lockstep slowdowns).  Pool only
    issues the fkin DMA.
  - r = sqrt(rho + eps) on ACT straight from the PE's rho.  Activation
    func sets thrash-free order: one resident table at a time, so the
    Scalar stream is [set3: sqrt,relu,square] -> load set2 -> [arctan,
    abs, copy ...].
  - theta: iy = phi0*P + Q with P,Q prep interleaved; Wy slot distances
    for all 8 bins in ONE batched TT against fp16 constants in fkin,
    one batched abs, two fold mins.  Hats relu'd on ACT (dedicated
    port): w = relu(1 - e) via scale=-1, bias=1.
  - single aliased PSUM tile [128,8,512]: 8 banks so all 8 matmul1
    groups accumulate independently; copies never stall the PE.
  - PE kept continuously busy (rho -> warm f32r matmuls on pe_s ->
    gpsi -> matmul1) to hold the 2.4GHz p-state (3us ramp); pe_s
    free-dim 480 >= 256 so f32r streams at 1 cycle/row.
  - a_t cell-major [128, cell, u, m]: contiguous DVE writes (the fast
    2x fp16 path needs large contiguous runs).
  - input DMAs issued bias-then-pein on the Scalar queue (observed to
    stream with ~0 start latency vs ~2us on sync).

Sharding: 8 cores; core c handles batch b = c//4, centers m0 = (c%4)*96.
"""

import math
import numpy as np

RADIUS = 1.5
B, M, N = 2, 384, 384
CI = CO = 8
M_LOC = 96          # centers per core
NCH = 3             # n-chunks of 128 (N = 384)
NCELL = 32          # cell = j*8 + b2  (j = radius 0..3, b2 = theta bin 0..7)
N_CORES = 8
PI = math.pi
SQB = 1e-5          # sqrt bias: guards f32r cancellation negatives in rho

CFG = dict(warm=32)

_module_cache = {}

# fkin fp16 column layout
FEAT0 = 0                 # 3 chunks * 17 (16 feat*mask, mask)
K20 = FEAT0 + 3 * 17      # 51: k2c, 4 j * 16 coy
LC8 = K20 + 64            # 115: Wy slot centers 1..8 as fp16
FKW = LC8 + 8             # 123

# bias f32 column layout: activation-bias constants, then coordinates
BIAS_VALS = [1.0, SQB, -0.5, -1.5, -2.5, -3.5, -2.0, -3.0, -6.0, -7.0]
BIAS_COL = {v: i for i, v in enumerate(BIAS_VALS)}
FX0 = len(BIAS_VALS)      # fx per chunk: 3 cols (then fy: 3 cols)
CX0 = FX0 + 6             # cx per m: 96 cols (then cy: 96)
BIASW = CX0 + 192


def _build_module(cfg):
    import concourse.bass as bass
    import concourse.bacc as bacc
    import concourse.mybir as mybir
    from concourse import tile

    dt = mybir.dt
    Alu = mybir.AluOpType
    Act = mybir.ActivationFunctionType

    nc = bacc.Bacc("TRN2", target_bir_lowering=False, debug=False,
                   num_devices=N_CORES)

    f32 = dt.float32
    f16 = dt.float16
    i16 = dt.int16
    W = cfg["warm"]

    # ------------- DRAM I/O -------------
    biasd = nc.dram_tensor("bias", [128, BIASW], f32,
                           kind="ExternalInput").ap()
    fkd = nc.dram_tensor("fkin", [128, FKW], f16, kind="ExternalInput").ap()
    outd = nc.dram_tensor("out", [M_LOC, 16], f32, kind="ExternalOutput").ap()

    with tile.TileContext(nc) as tc:
        with tc.tile_pool(name="p", bufs=1) as pool, \
             tc.tile_pool(name="ps", bufs=1, space="PSUM") as psum:

            V, S, G = nc.vector, nc.scalar, nc.gpsimd

            # ---------- SBUF tiles ----------
            fk_s = pool.tile([128, FKW], f16, tag="fk", name="fk_s")
            bias_s = pool.tile([128, BIASW], f32, tag="bias", name="bias_s")

            def bAP(v, parts=128):
                return bias_s[0:parts, BIAS_COL[v]:BIAS_COL[v] + 1]

            def t16(tag, shape=None):
                return pool.tile(shape or [128, NCH, M_LOC], f16, tag=tag,
                                 name=tag)

            relxy = t16("relxy", [128, 2, NCH, M_LOC])
            sq2 = t16("sq2", [128, 2, NCH, M_LOC])
            rhov = t16("rhov")
            axy = t16("axy", [128, 2, NCH, M_LOC])
            sgxy = t16("sgxy", [128, 2, NCH, M_LOC])
            mn = t16("mn")
            mx = pool.tile([128, NCH, M_LOC], f32, tag="mx", name="mx")
            rmx = pool.tile([128, NCH, M_LOC], f32, tag="rmx", name="rmx")
            ratio = t16("ratio")
            phi0 = t16("phi0")
            swp = t16("swp"); qa = t16("qa"); w2 = t16("w2"); Qt = t16("Qt")
            sxy = t16("sxy"); s1p = t16("s1p"); Pt = t16("Pt"); m1 = t16("m1")
            tphi = t16("tphi"); iy = t16("iy"); e9m = t16("e9m")
            r16 = t16("r16")
            ex = t16("ex", [128, 4, NCH, M_LOC])
            ey = t16("ey", [128, 8, NCH, M_LOC])
            wyt = t16("wyt", [128, 8, NCH, M_LOC])
            wxt = t16("wxt", [128, 4, NCH, M_LOC])
            wxa = t16("wxa", [128, 4, NCH, M_LOC])
            au = t16("au"); au2 = t16("au2")
            a_t = pool.tile([128, NCELL + 1, NCH, M_LOC], f16, tag="a_t",
                            name="a_t")
            gs_s = pool.tile([16, NCELL, M_LOC], f16, tag="gs", name="gs_s")
            gt = pool.tile([128, 4, M_LOC], f16, tag="gt", name="gt")
            psir = pool.tile([1, M_LOC], f32, tag="psir", name="psir")
            psit = pool.tile([M_LOC, 1], f32, tag="psit", name="psit")
            out_s = pool.tile([M_LOC, 16], f32, tag="outs", name="out_s")
            warm = pool.tile([1, 1], f32, tag="warm", name="warm")

            # ---------- PSUM: one aliased tile, manual bank layout ----------
            # bank0: rho | gh(0,0)        bank4: gh(2,0)
            # bank1: gh(1,0), o2t@400     bank5: gh(3,0)
            # bank2: warm | gh(2,1)       bank6: gh(0,1)
            # bank3: gpsi | gh(3,1)       bank7: gh(1,1)
            P = psum.tile([128, 8, 512], f32, tag="P", name="P")
            warm_ps = P[0:64, 2, 0:FKW]
            gpsi = P[0:1, 3, 0:M_LOC]
            o2t = P[0:M_LOC, 1, 400:416]
            GH_BANK = {(0, 0): 0, (1, 0): 1, (2, 0): 4, (3, 0): 5,
                       (0, 1): 6, (1, 1): 7, (2, 1): 2, (3, 1): 3}

            def ghv(j, h):
                return P[0:17, GH_BANK[(j, h)], 0:4 * M_LOC].rearrange(
                    "p (c m) -> p c m", c=4)

            # ---------- input DMAs (bias first: streams immediately) ------
            nc.scalar.dma_start(bias_s[:], biasd[:])
            nc.scalar.dma_start(fk_s[:], fkd[:])

            # act-set seed: Sqrt first-fits set 3 ({sqrt,relu,square,abs});
            # single switch to set 2 (arctan/abs/copy) before the arctan.
            V.memset(warm[:], 0.25)
            S.activation(warm[:], warm[:], Act.Sqrt)

            # ---------- PE p-state warm: cheap fp16 matmuls on fk_s ------
            for _ in range(W):
                nc.tensor.matmul(warm_ps[:], fk_s[:, 0:64], fk_s[:])

            # ---------- rel (f32 coords -> fp16) ----------
            f_b = bias_s[:, FX0:FX0 + 6, None].rearrange(
                "p (a u) o -> p a u o", a=2).to_broadcast(
                (128, 2, NCH, M_LOC))
            c_b = bias_s[:, None, CX0:CX0 + 192].rearrange(
                "p o (a m) -> p a o m", a=2).to_broadcast(
                (128, 2, NCH, M_LOC))
            V.tensor_tensor(relxy[:], f_b, c_b, Alu.subtract)
            # rho = relx^2 + rely^2 on DVE (fp16; keeps PE free)
            V.tensor_tensor(sq2[:], relxy[:], relxy[:], Alu.mult)
            V.tensor_tensor(rhov[:], sq2[:, 0], sq2[:, 1], Alu.add)

            # ---------- octant pieces (all DVE) ----------
            V.tensor_scalar(axy[:].bitcast(i16), relxy[:].bitcast(i16),
                            0x7FFF, None, Alu.bitwise_and)
            V.tensor_scalar(sgxy[:].bitcast(i16), relxy[:].bitcast(i16),
                            -32768, 0x3C00, Alu.bitwise_and, Alu.bitwise_or)
            ax, ay = axy[:, 0], axy[:, 1]
            sgx, sgy = sgxy[:, 0], sgxy[:, 1]
            V.tensor_tensor(mn[:], ax, ay, Alu.min)
            V.tensor_tensor(mx[:], ax, ay, Alu.max)
            V.tensor_tensor(swp[:], ay, ax, Alu.is_gt)
            V.reciprocal_approx_fast(rmx[:], mx[:])
            V.tensor_tensor(ratio[:], mn[:], rmx[:], Alu.mult)

            # theta prep (parallel with arctan):
            #   iy = phi0 * P + Q
            #   P = sxy * (4/pi) * (1 - 2*swp)
            #   Q = sxy * (2*swp - 2) + (2*sgy + 4.5)
            V.tensor_tensor(sxy[:], sgx, sgy, Alu.mult)
            V.tensor_scalar(s1p[:], swp[:], -8.0 / PI, 4.0 / PI,
                            Alu.mult, Alu.add)
            V.tensor_tensor(Pt[:], s1p[:], sxy[:], Alu.mult)
            V.tensor_scalar(qa[:], swp[:], 2.0, -2.0, Alu.mult, Alu.add)
            V.tensor_tensor(m1[:], qa[:], sxy[:], Alu.mult)
            V.tensor_scalar(w2[:], sgy, 2.0, 4.5, Alu.mult, Alu.add)
            V.tensor_tensor(Qt[:], m1[:], w2[:], Alu.add)

            # ---------- Scalar chain ----------
            S.activation(r16[:], rhov[:], Act.Sqrt, bias=bAP(SQB))
            S.activation(au[:], rhov[:], Act.Relu, bias=bAP(1.0), scale=-1.0)
            S.activation(au2[:], au[:], Act.Square)
            S.activation(phi0[:], ratio[:], Act.Arctan)
            # Wx slots: |4r - (0.5+j)|
            for j in range(4):
                S.activation(ex[:, j, :, :], r16[:], Act.Abs,
                             bias=bAP(-0.5 - j), scale=4.0)
            # wxt = relu(1 - ex) on ACT (dedicated port)
            S.activation(wxt[:], ex[:], Act.Relu, bias=bAP(1.0), scale=-1.0)

            # att -> a_t att cell
            V.tensor_tensor(a_t[:, NCELL, :, :], au2[:], au[:], Alu.mult)

            # ---------- theta tail + batched Wy slots ----------
            V.tensor_tensor(tphi[:], phi0[:], Pt[:], Alu.mult)
            V.tensor_tensor(iy[:], tphi[:], Qt[:], Alu.add)
            V.tensor_scalar(e9m[:], iy[:], -1.0, 9.0, Alu.mult, Alu.add)

            def vslot(l):
                d = ey[:, l - 1, :, :]
                V.tensor_scalar(d, iy[:], -float(l), None, Alu.add)
                V.tensor_scalar(d.bitcast(i16), d.bitcast(i16),
                                0x7FFF, None, Alu.bitwise_and)

            for l in (2, 3):
                S.activation(ey[:, l - 1, :, :], iy[:], Act.Abs,
                             bias=bAP(-float(l)))
            vslot(1)
            vslot(4)
            # circular folds: |iy-9| = 9-iy, |iy-0| = iy  (iy in [0.5, 8.5])
            V.tensor_tensor(ey[:, 0, :, :], ey[:, 0, :, :], e9m[:], Alu.min)
            S.activation(wyt[:, 0:4, :, :], ey[:, 0:4, :, :], Act.Relu,
                         bias=bAP(1.0), scale=-1.0)
            for l in (6, 7):
                S.activation(ey[:, l - 1, :, :], iy[:], Act.Abs,
                             bias=bAP(-float(l)))
            vslot(5)
            vslot(8)
            V.tensor_tensor(ey[:, 7, :, :], ey[:, 7, :, :], iy[:], Alu.min)
            S.activation(wyt[:, 4:8, :, :], ey[:, 4:8, :, :], Act.Relu,
                         bias=bAP(1.0), scale=-1.0)

            # wxa = wxt * att
            att_b = a_t[:, NCELL:NCELL + 1, :, :].to_broadcast(
                (128, 4, NCH, M_LOC))
            V.tensor_tensor(wxa[:], wxt[:], att_b, Alu.mult)

            # ---------- gpsi (psi) + warm chain ----------
            def feat_ap(u):
                return fk_s[:, FEAT0 + 17 * u:FEAT0 + 17 * (u + 1)]

            # psi via the mask column only -> partition 0 of bank 3
            for u in range(NCH):
                nc.tensor.matmul(gpsi[:],
                                 fk_s[:, FEAT0 + 17 * u + 16:
                                      FEAT0 + 17 * u + 17],
                                 a_t[:, NCELL, u, :],
                                 start=(u == 0), stop=(u == NCH - 1))
            # psi -> 1/psi -> [96, 1]
            V.tensor_scalar(psir[:], gpsi[0:1, :], 1e-35, None, Alu.max)
            V.reciprocal_approx_fast(psir[:], psir[:])
            nc.sync.dma_start(psit[:, 0:1], psir[0:1, :])

            # ---------- A cells + matmul1 + copies + transpose DMAs -------
            def wxa_b(j):
                return wxa[:, j:j + 1, :, :].to_broadcast(
                    (128, 8, NCH, M_LOC))

            def mm1(j, h):
                g = ghv(j, h)
                c0 = 8 * j + 4 * h
                for u in range(NCH):
                    nc.tensor.matmul(g[:], feat_ap(u),
                                     a_t[:, c0:c0 + 4, u, :],
                                     start=(u == 0), stop=(u == NCH - 1))

            def gcopy(j, h, eng):
                c0 = 8 * j + 4 * h
                src = ghv(j, h)[0:16]
                if eng is S:
                    S.activation(gs_s[:, c0:c0 + 4, :], src, Act.Copy)
                else:
                    eng.tensor_copy(gs_s[:, c0:c0 + 4, :], src)

            for j in range(4):
                # one 8-cell product per j on DVE (fast contiguous pattern)
                V.tensor_tensor(a_t[:, 8 * j:8 * j + 8, :, :], wxa_b(j),
                                wyt[:], Alu.mult)
                mm1(j, 0)
                mm1(j, 1)
                ceng = S if j < 3 else V
                gcopy(j, 0, ceng)
                gcopy(j, 1, ceng)
                nc.sync.dma_start(gt[:, j], gs_s[:, 8 * j:8 * j + 8, :])

            # ---------- matmul2 ----------
            for q in range(4):
                nc.tensor.matmul(o2t[:], gt[:, q, :],
                                 fk_s[:, K20 + 16 * q:K20 + 16 * (q + 1)],
                                 start=(q == 0), stop=(q == 3))

            # ---------- scale by 1/psi, store ----------
            V.tensor_scalar(out_s[:], o2t[:], psit[:, 0:1], None, Alu.mult)
            nc.scalar.dma_start(outd[:], out_s[:])

    nc.compile()
    return nc


def get_module(cfg=None):
    cfg = dict(CFG, **(cfg or {}))
    key = tuple(sorted((k, str(v)) for k, v in cfg.items()))
    if key not in _module_cache:
        _module_cache[key] = _build_module(cfg)
    return _module_cache[key]


def make_in_maps(field, center, field_feat, field_mask, kernel, cfg=None):
    """Host-side shard + layout prep. Returns list of 8 in_maps."""
    field = np.asarray(field, np.float32)
    center = np.asarray(center, np.float32)
    feat = np.asarray(field_feat, np.float32)
    mask = np.asarray(field_mask, np.float32)
    ker = np.asarray(kernel, np.float32)

    # kk[cell=(th*4+r), f=(ci,x), coy=(co,y)]
    kk = ker.transpose(3, 2, 1, 5, 0, 4).reshape(NCELL, 16, 16)
    # k2c[p=(f*8+b2), j, coy] = kk[b2*4+j, f]
    k2c = np.zeros((128, 4, 16), np.float32)
    for bth in range(8):
        for j in range(4):
            for f in range(16):
                k2c[f * 8 + bth, j] = kk[bth * 4 + j, f]

    in_maps = []
    for c in range(N_CORES):
        b, blk = divmod(c, 4)
        m0 = blk * M_LOC
        cx = center[b, m0:m0 + M_LOC, 0] / RADIUS   # [96]
        cy = center[b, m0:m0 + M_LOC, 1] / RADIUS
        fx = (field[b, :, 0] / RADIUS).reshape(NCH, 128)  # [3, 128]
        fy = (field[b, :, 1] / RADIUS).reshape(NCH, 128)
        biasf = np.zeros((128, BIASW), np.float32)
        biasf[:, 0:len(BIAS_VALS)] = np.array(BIAS_VALS, np.float32)
        biasf[:, FX0:FX0 + 3] = fx.T
        biasf[:, FX0 + 3:FX0 + 6] = fy.T
        biasf[:, CX0:CX0 + 96] = cx
        biasf[:, CX0 + 96:CX0 + 192] = cy

        fkin = np.zeros((128, FKW), np.float32)
        fm = feat[b].reshape(N, 16) * mask[b]
        fcols = np.concatenate([fm, mask[b]], axis=1)        # [N, 17]
        fkin[:, FEAT0:FEAT0 + 51] = (
            fcols.reshape(NCH, 128, 17).transpose(1, 0, 2).reshape(128, 51))
        fkin[:, K20:K20 + 64] = k2c.reshape(128, 64)
        fkin[:, LC8:LC8 + 8] = np.arange(1, 9, dtype=np.float32)

        in_maps.append({
            "bias": biasf,
            "fkin": fkin.astype(np.float16),
        })
    return in_maps


def unshard(results):
    out = np.zeros((B, M, CO, 2), np.float32)
    for c in range(N_CORES):
        b, blk = divmod(c, 4)
        m0 = blk * M_LOC
        out[b, m0:m0 + M_LOC] = results[c]["out"].reshape(M_LOC, CO, 2)
    return out


def kernel(field, center, field_feat, field_mask, kernel):
    from concourse.bass_utils import run_bass_kernel_spmd
    nc = get_module()
    in_maps = make_in_maps(field, center, field_feat, field_mask, kernel)
    res = run_bass_kernel_spmd(nc, in_maps, core_ids=list(range(N_CORES)))
    return unshard(res.results)
